# revision 1
# baseline (speedup 1.0000x reference)
"""MoE gate (DeepSeek-V3 noaux_tc routing) on 8 Trainium2 NeuronCores.

Strategy: sequence-parallel — shard the 16384-token axis across 8 cores
(2048 tokens each), replicate the [256,7168] gate weight.

Numerics: router matmul done as 3-term bf16 split (hi*hi + hi*lo + lo*hi)
which measures ~5e-6 rms vs fp64 (close to fp32's own rounding noise).
fp32r was measured at 1.8e-4 rms — too coarse for top-k boundaries.
Top-k uses the DVE max8/max_index instructions (exact, stable-index).
"""
import sys
import os

sys.path.insert(0, "/opt/trn_rl_repo")

import numpy as np
import ml_dtypes

SEQ = 16384
HID = 7168
EXP = 256
N_CORES = 8
TOK = SEQ // N_CORES          # 2048 tokens per core
P = 128                       # partition dim / token tile
TILES = TOK // P              # 16
KT = HID // P                 # 56 contraction chunks
NG = 8                        # groups
GS = EXP // NG                # 32 experts per group
SCALE = 2.5

_CACHE = {}
LAST_RESULTS = None


def _build_program():
    import concourse.mybir as mybir
    import concourse.tile as tile
    from concourse import bacc

    nc = bacc.Bacc("TRN2", target_bir_lowering=False, debug=False,
                   num_devices=N_CORES)

    bf16 = mybir.dt.bfloat16
    f32 = mybir.dt.float32

    d_hhi = nc.dram_tensor("h_hi", [HID, TOK], bf16, kind="ExternalInput").ap()
    d_hlo = nc.dram_tensor("h_lo", [HID, TOK], bf16, kind="ExternalInput").ap()
    d_wcat = nc.dram_tensor("w_cat", [HID, 2 * EXP], bf16, kind="ExternalInput").ap()
    d_bias = nc.dram_tensor("bias_rep", [P, EXP], f32, kind="ExternalInput").ap()
    d_iota = nc.dram_tensor("iota_rep", [P, EXP], f32, kind="ExternalInput").ap()
    d_ow = nc.dram_tensor("out_w", [TOK, 8], f32, kind="ExternalOutput").ap()
    d_oi = nc.dram_tensor("out_i", [TOK, 8], mybir.dt.int32, kind="ExternalOutput").ap()

    # DRAM access patterns: [HID=(k p), TOK=(t m)] -> [p, k, (t), m]
    hhi_p = d_hhi.rearrange("(k p) (t m) -> p k t m", p=P, m=P)
    hlo_p = d_hlo.rearrange("(k p) (t m) -> p k t m", p=P, m=P)
    wcat_p = d_wcat.rearrange("(k p) e -> p k e", p=P)

    X = mybir.AxisListType.X
    op = mybir.AluOpType

    with tile.TileContext(nc) as tc:
        with tc.tile_pool(name="wpool", bufs=1) as wp, \
             tc.tile_pool(name="hpool", bufs=3) as hp, \
             tc.tile_pool(name="spool", bufs=2) as sp, \
             tc.tile_pool(name="smalls", bufs=2) as smp, \
             tc.tile_pool(name="opool", bufs=3) as outp, \
             tc.tile_pool(name="psum", bufs=2, space="PSUM") as pp:

            # --- persistent weights / bias ---
            w_cat = wp.tile([P, KT * 2 * EXP], bf16, tag="w_cat")
            w_cat3 = w_cat[:].rearrange("p (k e) -> p k e", e=2 * EXP)
            nc.sync.dma_start(out=w_cat3, in_=wcat_p)
            bias_t = wp.tile([P, EXP], f32, tag="bias")
            nc.sync.dma_start(out=bias_t[:], in_=d_bias)
            iota_t = wp.tile([P, EXP], f32, tag="iota")
            nc.sync.dma_start(out=iota_t[:], in_=d_iota)

            for t in range(TILES):
                # --- load hidden tile (transposed layout, hi/lo bf16) ---
                h_hi = hp.tile([P, KT * P], bf16, tag="h_hi")
                h_lo = hp.tile([P, KT * P], bf16, tag="h_lo")
                nc.sync.dma_start(
                    out=h_hi[:].rearrange("p (k m) -> p k m", m=P),
                    in_=hhi_p[:, :, t, :])
                nc.sync.dma_start(
                    out=h_lo[:].rearrange("p (k m) -> p k m", m=P),
                    in_=hlo_p[:, :, t, :])

                # --- matmul: psum[:, :256] = hi*w_hi (+ lo*w_hi),
                #             psum[:, 256:] = hi*w_lo ---
                ps = pp.tile([P, 2 * EXP], f32, tag="ps")
                for k in range(KT):
                    hs = slice(k * P, (k + 1) * P)
                    nc.tensor.matmul(
                        ps[:, :], h_hi[:, hs], w_cat[:, k * 2 * EXP:(k + 1) * 2 * EXP],
                        start=(k == 0), stop=False, skip_group_check=True)
                    nc.tensor.matmul(
                        ps[:, 0:EXP], h_lo[:, hs],
                        w_cat[:, k * 2 * EXP: k * 2 * EXP + EXP],
                        start=False, stop=(k == KT - 1), skip_group_check=True)

                # --- logits = left + right; scores = sigmoid(logits) ---
                # (DVE can read only one PSUM operand; bounce right half via ACT)
                xterm = sp.tile([P, EXP], f32, tag="xterm")
                nc.scalar.copy(xterm[:], ps[:, EXP:2 * EXP])
                logits = sp.tile([P, EXP], f32, tag="logits")
                nc.vector.tensor_add(logits[:], ps[:, 0:EXP], xterm[:])
                scores = sp.tile([P, EXP], f32, tag="scores")
                nc.scalar.activation(scores[:], logits[:],
                                     mybir.ActivationFunctionType.Sigmoid)
                s4c = sp.tile([P, EXP], f32, tag="s4c")
                nc.vector.tensor_add(s4c[:], scores[:], bias_t[:])
                s4c3 = s4c[:].rearrange("p (g e) -> p g e", e=GS)

                # --- group scores: sum of top-2 per group of 32 ---
                gmax1 = smp.tile([P, NG], f32, tag="gmax1")
                nc.vector.reduce_max(gmax1[:], s4c3, axis=X)
                eq = sp.tile([P, EXP], f32, tag="eq")
                nc.vector.tensor_tensor(
                    eq[:].rearrange("p (g e) -> p g e", e=GS), s4c3,
                    gmax1[:].to_broadcast([P, NG, GS]), op=op.is_equal)
                masked = sp.tile([P, EXP], f32, tag="masked")
                nc.vector.scalar_tensor_tensor(
                    out=masked[:], in0=eq[:], scalar=-1e30, in1=s4c[:],
                    op0=op.mult, op1=op.add)
                gmax2 = smp.tile([P, NG], f32, tag="gmax2")
                nc.vector.reduce_max(
                    gmax2[:], masked[:].rearrange("p (g e) -> p g e", e=GS), axis=X)
                gsum = smp.tile([P, NG], f32, tag="gsum")
                nc.vector.tensor_add(gsum[:], gmax1[:], gmax2[:])

                # --- top-4 groups -> expert mask -> masked scores ---
                gsort = smp.tile([P, 8], f32, tag="gsort")
                nc.vector.max(out=gsort[:], in_=gsum[:])
                gmask = smp.tile([P, NG], f32, tag="gmask")
                nc.vector.tensor_scalar(
                    out=gmask[:], in0=gsum[:], scalar1=gsort[:, 3:4], scalar2=None,
                    op0=op.is_ge)
                tmp = sp.tile([P, EXP], f32, tag="tmp")
                nc.vector.tensor_tensor(
                    tmp[:].rearrange("p (g e) -> p g e", e=GS), s4c3,
                    gmask[:].to_broadcast([P, NG, GS]), op=op.mult)

                # --- top-8 over masked corrected scores ---
                v8 = outp.tile([P, 8], f32, tag="v8")
                nc.vector.max(out=v8[:], in_=tmp[:])
                idx8 = outp.tile([P, 8], mybir.dt.uint32, tag="idx8")
                nc.vector.max_index(out=idx8[:], in_max=v8[:], in_values=tmp[:])

                # --- gather uncorrected scores at the top-8 positions ---
                # keyed on the (unique) index, not the value: two experts can
                # have bitwise-equal corrected scores, which would double-match
                idxf = outp.tile([P, 8], f32, tag="idxf")
                nc.vector.tensor_copy(idxf[:], idx8[:])
                wsel = outp.tile([P, 8], f32, tag="wsel")
                scratch = sp.tile([P, EXP], f32, tag="scratch")
                for kk in range(8):
                    nc.vector.scalar_tensor_tensor(
                        out=scratch[:], in0=iota_t[:], scalar=idxf[:, kk:kk + 1],
                        in1=scores[:], op0=op.is_equal, op1=op.mult,
                        accum_out=wsel[:, kk:kk + 1])

                # --- renormalize * 2.5 ---
                denom = smp.tile([P, 1], f32, tag="denom")
                nc.vector.reduce_sum(denom[:], wsel[:], axis=X)
                nc.vector.tensor_scalar_add(denom[:], denom[:], 1e-20)
                recip = smp.tile([P, 1], f32, tag="recip")
                nc.vector.reciprocal(recip[:], denom[:])
                wfin = outp.tile([P, 8], f32, tag="wfin")
                nc.vector.tensor_scalar(
                    out=wfin[:], in0=wsel[:], scalar1=recip[:, 0:1], scalar2=SCALE,
                    op0=op.mult, op1=op.mult)

                nc.sync.dma_start(out=d_ow[t * P:(t + 1) * P, :], in_=wfin[:])
                nc.sync.dma_start(out=d_oi[t * P:(t + 1) * P, :],
                                  in_=idx8[:].bitcast(mybir.dt.int32))

    nc.compile()
    return nc


def _get_program():
    if "nc" not in _CACHE:
        _CACHE["nc"] = _build_program()
    return _CACHE["nc"]


def _prepare_in_maps(hidden_states, weight, e_score_correction_bias):
    h = np.asarray(hidden_states, dtype=np.float32)
    w = np.asarray(weight, dtype=np.float32)
    b = np.asarray(e_score_correction_bias, dtype=np.float32)

    bf16 = ml_dtypes.bfloat16
    hT = np.ascontiguousarray(h.T)                      # [HID, SEQ]
    h_hi = hT.astype(bf16)
    h_lo = (hT - h_hi.astype(np.float32)).astype(bf16)
    wT = np.ascontiguousarray(w.T)                      # [HID, EXP]
    w_hi = wT.astype(bf16)
    w_lo = (wT - w_hi.astype(np.float32)).astype(bf16)
    w_cat = np.ascontiguousarray(np.concatenate([w_hi, w_lo], axis=1))
    bias_rep = np.ascontiguousarray(np.broadcast_to(b[None, :], (P, EXP)))
    iota_rep = np.ascontiguousarray(
        np.broadcast_to(np.arange(EXP, dtype=np.float32)[None, :], (P, EXP)))

    in_maps = []
    for c in range(N_CORES):
        sl = slice(c * TOK, (c + 1) * TOK)
        in_maps.append({
            "h_hi": np.ascontiguousarray(h_hi[:, sl]),
            "h_lo": np.ascontiguousarray(h_lo[:, sl]),
            "w_cat": w_cat,
            "bias_rep": bias_rep,
            "iota_rep": iota_rep,
        })
    return in_maps


def kernel(hidden_states, weight, e_score_correction_bias):
    global LAST_RESULTS
    from concourse.bass_utils import run_bass_kernel_spmd

    nc = _get_program()
    in_maps = _prepare_in_maps(hidden_states, weight, e_score_correction_bias)

    trace = bool(int(os.environ.get("KERNEL_TRACE", "0")))
    res = run_bass_kernel_spmd(nc, in_maps, core_ids=list(range(N_CORES)),
                               trace=trace)
    LAST_RESULTS = res

    topk_w = np.concatenate([res.results[c]["out_w"] for c in range(N_CORES)], axis=0)
    topk_i = np.concatenate([res.results[c]["out_i"] for c in range(N_CORES)], axis=0)
    return topk_w, topk_i



# revision 2
# speedup vs baseline: 1.2076x; 1.2076x over previous
"""MoE gate (DeepSeek-V3 noaux_tc routing) on 8 Trainium2 NeuronCores.

Strategy: sequence-parallel — shard the 16384-token axis across 8 cores
(2048 tokens each), replicate the [256,7168] gate weight.

Numerics: router matmul done as 3-term bf16 split (hi*hi + hi*lo + lo*hi)
which measures ~5e-6 rms vs fp64 (close to fp32's own rounding noise).
fp32r was measured at 1.8e-4 rms — too coarse for top-k boundaries.
Top-k uses the DVE max8/max_index instructions (exact, stable-index).

v2: DRAM layout packed host-side as [partition][tile][hi/lo][k][m] so each
per-tile DMA is one contiguous 28KB run per partition (was 256B runs —
descriptor-rate-bound, stalling the PE). Weight DMA split in two so the
first k-chunks' matmuls start before the full weight lands.
"""
import sys
import os

sys.path.insert(0, "/opt/trn_rl_repo")

import numpy as np
import ml_dtypes

SEQ = 16384
HID = 7168
EXP = 256
N_CORES = 8
TOK = SEQ // N_CORES          # 2048 tokens per core
P = 128                       # partition dim / token tile
TILES = TOK // P              # 16
KT = HID // P                 # 56 contraction chunks
KHALF = KT // 2               # 28 (weight DMA split point)
NG = 8                        # groups
GS = EXP // NG                # 32 experts per group
SCALE = 2.5

_CACHE = {}
LAST_RESULTS = None


def _build_program():
    import concourse.mybir as mybir
    import concourse.tile as tile
    from concourse import bacc

    nc = bacc.Bacc("TRN2", target_bir_lowering=False, debug=False,
                   num_devices=N_CORES)

    bf16 = mybir.dt.bfloat16
    f32 = mybir.dt.float32

    # h_cat layout: [p][t][s][k][m], s=0 is bf16-hi, s=1 is bf16-lo
    d_hcat = nc.dram_tensor("h_cat", [P, TILES * 2 * KT * P], bf16,
                            kind="ExternalInput").ap()
    # w_cat layout: [p][k][e(512)], e 0:256 = w_hi, 256:512 = w_lo
    d_wcat = nc.dram_tensor("w_cat", [P, KT * 2 * EXP], bf16,
                            kind="ExternalInput").ap()
    d_bias = nc.dram_tensor("bias_rep", [P, EXP], f32, kind="ExternalInput").ap()
    d_iota = nc.dram_tensor("iota_rep", [P, EXP], f32, kind="ExternalInput").ap()
    d_ow = nc.dram_tensor("out_w", [TOK, 8], f32, kind="ExternalOutput").ap()
    d_oi = nc.dram_tensor("out_i", [TOK, 8], mybir.dt.int32, kind="ExternalOutput").ap()

    hcat_t = d_hcat.rearrange("p (t r) -> p t r", t=TILES)
    wcat_h = d_wcat.rearrange("p (h r) -> p h r", h=2)

    X = mybir.AxisListType.X
    op = mybir.AluOpType

    with tile.TileContext(nc) as tc:
        with tc.tile_pool(name="wpool", bufs=1) as wp, \
             tc.tile_pool(name="hpool", bufs=3) as hp, \
             tc.tile_pool(name="spool", bufs=2) as sp, \
             tc.tile_pool(name="smalls", bufs=2) as smp, \
             tc.tile_pool(name="opool", bufs=3) as outp, \
             tc.tile_pool(name="psum", bufs=2, space="PSUM") as pp:

            # --- persistent weights / bias (two halves so k<28 starts early)
            w_a = wp.tile([P, KHALF * 2 * EXP], bf16, tag="w_a")
            nc.sync.dma_start(out=w_a[:], in_=wcat_h[:, 0])
            w_b = wp.tile([P, KHALF * 2 * EXP], bf16, tag="w_b")
            nc.sync.dma_start(out=w_b[:], in_=wcat_h[:, 1])
            bias_t = wp.tile([P, EXP], f32, tag="bias")
            nc.sync.dma_start(out=bias_t[:], in_=d_bias)
            iota_t = wp.tile([P, EXP], f32, tag="iota")
            nc.sync.dma_start(out=iota_t[:], in_=d_iota)

            for t in range(TILES):
                # --- load hidden tile: one DMA, contiguous per partition ---
                hc = hp.tile([P, 2 * KT * P], bf16, tag="hcat")
                nc.sync.dma_start(out=hc[:], in_=hcat_t[:, t])
                hc4 = hc[:].rearrange("p (s k m) -> p s k m", s=2, m=P)

                # --- matmul: psum[:, :256] = hi*w_hi (+ lo*w_hi),
                #             psum[:, 256:] = hi*w_lo ---
                ps = pp.tile([P, 2 * EXP], f32, tag="ps")
                for k in range(KT):
                    wt = w_a if k < KHALF else w_b
                    kk = k if k < KHALF else k - KHALF
                    ws = kk * 2 * EXP
                    nc.tensor.matmul(
                        ps[:, :], hc4[:, 0, k], wt[:, ws:ws + 2 * EXP],
                        start=(k == 0), stop=False, skip_group_check=True)
                    nc.tensor.matmul(
                        ps[:, 0:EXP], hc4[:, 1, k], wt[:, ws:ws + EXP],
                        start=False, stop=(k == KT - 1), skip_group_check=True)

                # --- logits = left + right; scores = sigmoid(logits) ---
                # (DVE can read only one PSUM operand; bounce right half via ACT)
                xterm = sp.tile([P, EXP], f32, tag="xterm")
                nc.scalar.copy(xterm[:], ps[:, EXP:2 * EXP])
                logits = sp.tile([P, EXP], f32, tag="logits")
                nc.vector.tensor_add(logits[:], ps[:, 0:EXP], xterm[:])
                scores = sp.tile([P, EXP], f32, tag="scores")
                nc.scalar.activation(scores[:], logits[:],
                                     mybir.ActivationFunctionType.Sigmoid)
                s4c = sp.tile([P, EXP], f32, tag="s4c")
                nc.vector.tensor_add(s4c[:], scores[:], bias_t[:])
                s4c3 = s4c[:].rearrange("p (g e) -> p g e", e=GS)

                # --- group scores: sum of top-2 per group of 32 ---
                gmax1 = smp.tile([P, NG], f32, tag="gmax1")
                nc.vector.reduce_max(gmax1[:], s4c3, axis=X)
                eq = sp.tile([P, EXP], f32, tag="eq")
                nc.vector.tensor_tensor(
                    eq[:].rearrange("p (g e) -> p g e", e=GS), s4c3,
                    gmax1[:].to_broadcast([P, NG, GS]), op=op.is_equal)
                masked = sp.tile([P, EXP], f32, tag="masked")
                nc.vector.scalar_tensor_tensor(
                    out=masked[:], in0=eq[:], scalar=-1e30, in1=s4c[:],
                    op0=op.mult, op1=op.add)
                gmax2 = smp.tile([P, NG], f32, tag="gmax2")
                nc.vector.reduce_max(
                    gmax2[:], masked[:].rearrange("p (g e) -> p g e", e=GS), axis=X)
                gsum = smp.tile([P, NG], f32, tag="gsum")
                nc.vector.tensor_add(gsum[:], gmax1[:], gmax2[:])

                # --- top-4 groups -> expert mask -> masked scores ---
                gsort = smp.tile([P, 8], f32, tag="gsort")
                nc.vector.max(out=gsort[:], in_=gsum[:])
                gmask = smp.tile([P, NG], f32, tag="gmask")
                nc.vector.tensor_scalar(
                    out=gmask[:], in0=gsum[:], scalar1=gsort[:, 3:4], scalar2=None,
                    op0=op.is_ge)
                tmp = sp.tile([P, EXP], f32, tag="tmp")
                nc.vector.tensor_tensor(
                    tmp[:].rearrange("p (g e) -> p g e", e=GS), s4c3,
                    gmask[:].to_broadcast([P, NG, GS]), op=op.mult)

                # --- top-8 over masked corrected scores ---
                v8 = outp.tile([P, 8], f32, tag="v8")
                nc.vector.max(out=v8[:], in_=tmp[:])
                idx8 = outp.tile([P, 8], mybir.dt.uint32, tag="idx8")
                nc.vector.max_index(out=idx8[:], in_max=v8[:], in_values=tmp[:])

                # --- gather uncorrected scores at the top-8 positions ---
                # keyed on the (unique) index, not the value: two experts can
                # have bitwise-equal corrected scores, which would double-match
                idxf = outp.tile([P, 8], f32, tag="idxf")
                nc.vector.tensor_copy(idxf[:], idx8[:])
                wsel = outp.tile([P, 8], f32, tag="wsel")
                scratch = sp.tile([P, EXP], f32, tag="scratch")
                for kk in range(8):
                    nc.vector.scalar_tensor_tensor(
                        out=scratch[:], in0=iota_t[:], scalar=idxf[:, kk:kk + 1],
                        in1=scores[:], op0=op.is_equal, op1=op.mult,
                        accum_out=wsel[:, kk:kk + 1])

                # --- renormalize * 2.5 ---
                denom = smp.tile([P, 1], f32, tag="denom")
                nc.vector.reduce_sum(denom[:], wsel[:], axis=X)
                nc.vector.tensor_scalar_add(denom[:], denom[:], 1e-20)
                recip = smp.tile([P, 1], f32, tag="recip")
                nc.vector.reciprocal(recip[:], denom[:])
                wfin = outp.tile([P, 8], f32, tag="wfin")
                nc.vector.tensor_scalar(
                    out=wfin[:], in0=wsel[:], scalar1=recip[:, 0:1], scalar2=SCALE,
                    op0=op.mult, op1=op.mult)

                nc.sync.dma_start(out=d_ow[t * P:(t + 1) * P, :], in_=wfin[:])
                nc.sync.dma_start(out=d_oi[t * P:(t + 1) * P, :],
                                  in_=idx8[:].bitcast(mybir.dt.int32))

    nc.compile()
    return nc


def _get_program():
    if "nc" not in _CACHE:
        _CACHE["nc"] = _build_program()
    return _CACHE["nc"]


def _prepare_in_maps(hidden_states, weight, e_score_correction_bias):
    h = np.asarray(hidden_states, dtype=np.float32)
    w = np.asarray(weight, dtype=np.float32)
    b = np.asarray(e_score_correction_bias, dtype=np.float32)

    bf16 = ml_dtypes.bfloat16
    hT = np.ascontiguousarray(h.T)                      # [HID, SEQ]
    h_hi = hT.astype(bf16)
    h_lo = (hT - h_hi.astype(np.float32)).astype(bf16)
    # pack to [p, T_global, s, k, m]: value = h_s[k*128+p, T*128+m]
    n_gt = SEQ // P                                     # 128 global tiles
    hcat = np.empty((P, n_gt, 2, KT, P), dtype=bf16)
    hcat[:, :, 0] = h_hi.reshape(KT, P, n_gt, P).transpose(1, 2, 0, 3)
    hcat[:, :, 1] = h_lo.reshape(KT, P, n_gt, P).transpose(1, 2, 0, 3)

    wT = np.ascontiguousarray(w.T)                      # [HID, EXP]
    w_hi = wT.astype(bf16)
    w_lo = (wT - w_hi.astype(np.float32)).astype(bf16)
    w_cat = np.concatenate([w_hi, w_lo], axis=1)        # [HID, 512]
    # pack to [p, k, e]
    w_cat = np.ascontiguousarray(
        w_cat.reshape(KT, P, 2 * EXP).transpose(1, 0, 2).reshape(P, KT * 2 * EXP))
    bias_rep = np.ascontiguousarray(np.broadcast_to(b[None, :], (P, EXP)))
    iota_rep = np.ascontiguousarray(
        np.broadcast_to(np.arange(EXP, dtype=np.float32)[None, :], (P, EXP)))

    in_maps = []
    for c in range(N_CORES):
        sl = slice(c * TILES, (c + 1) * TILES)
        in_maps.append({
            "h_cat": hcat[:, sl].reshape(P, TILES * 2 * KT * P),
            "w_cat": w_cat,
            "bias_rep": bias_rep,
            "iota_rep": iota_rep,
        })
    return in_maps


def kernel(hidden_states, weight, e_score_correction_bias):
    global LAST_RESULTS
    from concourse.bass_utils import run_bass_kernel_spmd

    nc = _get_program()
    in_maps = _prepare_in_maps(hidden_states, weight, e_score_correction_bias)

    trace = bool(int(os.environ.get("KERNEL_TRACE", "0")))
    res = run_bass_kernel_spmd(nc, in_maps, core_ids=list(range(N_CORES)),
                               trace=trace)
    LAST_RESULTS = res

    topk_w = np.concatenate([res.results[c]["out_w"] for c in range(N_CORES)], axis=0)
    topk_i = np.concatenate([res.results[c]["out_i"] for c in range(N_CORES)], axis=0)
    return topk_w, topk_i


# revision 5
# speedup vs baseline: 1.2242x; 1.0137x over previous
"""MoE gate (DeepSeek-V3 noaux_tc routing) on 8 Trainium2 NeuronCores.

Strategy: sequence-parallel — shard the 16384-token axis across 8 cores
(2048 tokens each), replicate the [256,7168] gate weight.

Numerics: router matmul done as 3-term bf16 split (hi*hi + hi*lo + lo*hi)
which measures ~5e-6 rms vs fp64 (close to fp32's own rounding noise).
fp32r was measured at 1.8e-4 rms — too coarse for top-k boundaries.
Top-k uses the DVE max8/max_index instructions (exact, stable-index).

v2: DRAM layout packed host-side as [partition][tile][hi/lo][k][m] so each
per-tile DMA is one contiguous 28KB run per partition (was 256B runs —
descriptor-rate-bound, stalling the PE). Weight DMA split in two so the
first k-chunks' matmuls start before the full weight lands.
"""
import sys
import os

sys.path.insert(0, "/opt/trn_rl_repo")

import numpy as np
import ml_dtypes

SEQ = 16384
HID = 7168
EXP = 256
N_CORES = 8
TOK = SEQ // N_CORES          # 2048 tokens per core
P = 128                       # partition dim / token tile
TILES = TOK // P              # 16
KT = HID // P                 # 56 contraction chunks
KHALF = KT // 2               # 28 (weight DMA split point)
NG = 8                        # groups
GS = EXP // NG                # 32 experts per group
SCALE = 2.5

_CACHE = {}
LAST_RESULTS = None


def _build_program():
    import concourse.mybir as mybir
    import concourse.tile as tile
    from concourse import bacc

    nc = bacc.Bacc("TRN2", target_bir_lowering=False, debug=False,
                   num_devices=N_CORES)

    bf16 = mybir.dt.bfloat16
    f32 = mybir.dt.float32

    # h_cat layout: [p][t][k][s][m], s=0 is bf16-hi, s=1 is bf16-lo
    # (s inside k so a k-range is one contiguous run per partition)
    d_hcat = nc.dram_tensor("h_cat", [P, TILES * 2 * KT * P], bf16,
                            kind="ExternalInput").ap()
    # w_cat layout: [p][k][e(512)], e 0:256 = w_hi, 256:512 = w_lo
    d_wcat = nc.dram_tensor("w_cat", [P, KT * 2 * EXP], bf16,
                            kind="ExternalInput").ap()
    d_bias = nc.dram_tensor("bias_rep", [P, EXP], f32, kind="ExternalInput").ap()
    d_iota = nc.dram_tensor("iota_rep", [P, EXP], f32, kind="ExternalInput").ap()
    d_ow = nc.dram_tensor("out_w", [TOK, 8], f32, kind="ExternalOutput").ap()
    d_oi = nc.dram_tensor("out_i", [TOK, 8], mybir.dt.int32, kind="ExternalOutput").ap()

    # [p, t, half(2), chunk-per-half] for the split h DMAs
    hcat_t = d_hcat.rearrange("p (t h r) -> p t h r", t=TILES, h=2)
    WCH = 8                       # k's per weight chunk
    NWCH = KT // WCH              # 7 weight chunks
    wcat_c = d_wcat.rearrange("p (c r) -> p c r", c=NWCH)

    X = mybir.AxisListType.X
    op = mybir.AluOpType

    with tile.TileContext(nc) as tc:
        with tc.tile_pool(name="wpool", bufs=1) as wp, \
             tc.tile_pool(name="hpool", bufs=3) as hp, \
             tc.tile_pool(name="spool", bufs=2) as sp, \
             tc.tile_pool(name="smalls", bufs=2) as smp, \
             tc.tile_pool(name="opool", bufs=3) as outp, \
             tc.tile_pool(name="psum", bufs=2, space="PSUM") as pp:

            # --- persistent weights / bias (7 chunks of 8 k's so the first
            # matmuls start after ~1MB instead of the full 7.3MB) ---
            w_c = []
            for c in range(NWCH):
                wt = wp.tile([P, WCH * 2 * EXP], bf16, tag=f"w_c{c}")
                nc.sync.dma_start(out=wt[:], in_=wcat_c[:, c])
                w_c.append(wt)
            bias_t = wp.tile([P, EXP], f32, tag="bias")
            nc.sync.dma_start(out=bias_t[:], in_=d_bias)
            iota_t = wp.tile([P, EXP], f32, tag="iota")
            nc.sync.dma_start(out=iota_t[:], in_=d_iota)

            for t in range(TILES):
                # --- load hidden tile: two DMAs (k<28, k>=28), contiguous ---
                hc_a = hp.tile([P, KHALF * 2 * P], bf16, tag="hcat_a")
                nc.sync.dma_start(out=hc_a[:], in_=hcat_t[:, t, 0])
                hc_b = hp.tile([P, KHALF * 2 * P], bf16, tag="hcat_b")
                nc.sync.dma_start(out=hc_b[:], in_=hcat_t[:, t, 1])
                hc4a = hc_a[:].rearrange("p (k s m) -> p k s m", s=2, m=P)
                hc4b = hc_b[:].rearrange("p (k s m) -> p k s m", s=2, m=P)

                # --- matmul: psum[:, :256] = hi*w_hi (+ lo*w_hi),
                #             psum[:, 256:] = hi*w_lo ---
                ps = pp.tile([P, 2 * EXP], f32, tag="ps")
                for k in range(KT):
                    hck = hc4a if k < KHALF else hc4b
                    kk = k if k < KHALF else k - KHALF
                    wt = w_c[k // WCH]
                    ws = (k % WCH) * 2 * EXP
                    nc.tensor.matmul(
                        ps[:, :], hck[:, kk, 0], wt[:, ws:ws + 2 * EXP],
                        start=(k == 0), stop=False, skip_group_check=True)
                    nc.tensor.matmul(
                        ps[:, 0:EXP], hck[:, kk, 1], wt[:, ws:ws + EXP],
                        start=False, stop=(k == KT - 1), skip_group_check=True)

                # --- logits = left + right; scores = sigmoid(logits) ---
                # (DVE can read only one PSUM operand; bounce right half via ACT)
                xterm = sp.tile([P, EXP], f32, tag="xterm")
                nc.scalar.copy(xterm[:], ps[:, EXP:2 * EXP])
                logits = sp.tile([P, EXP], f32, tag="logits")
                nc.vector.tensor_add(logits[:], ps[:, 0:EXP], xterm[:])
                scores = sp.tile([P, EXP], f32, tag="scores")
                nc.scalar.activation(scores[:], logits[:],
                                     mybir.ActivationFunctionType.Sigmoid)
                s4c = sp.tile([P, EXP], f32, tag="s4c")
                nc.vector.tensor_add(s4c[:], scores[:], bias_t[:])
                s4c3 = s4c[:].rearrange("p (g e) -> p g e", e=GS)

                # --- group scores: sum of top-2 per group of 32 ---
                gmax1 = smp.tile([P, NG], f32, tag="gmax1")
                nc.vector.reduce_max(gmax1[:], s4c3, axis=X)
                eq = sp.tile([P, EXP], f32, tag="eq")
                nc.vector.tensor_tensor(
                    eq[:].rearrange("p (g e) -> p g e", e=GS), s4c3,
                    gmax1[:].to_broadcast([P, NG, GS]), op=op.is_equal)
                masked = sp.tile([P, EXP], f32, tag="masked")
                nc.vector.scalar_tensor_tensor(
                    out=masked[:], in0=eq[:], scalar=-1e30, in1=s4c[:],
                    op0=op.mult, op1=op.add)
                gmax2 = smp.tile([P, NG], f32, tag="gmax2")
                nc.vector.reduce_max(
                    gmax2[:], masked[:].rearrange("p (g e) -> p g e", e=GS), axis=X)
                gsum = smp.tile([P, NG], f32, tag="gsum")
                nc.vector.tensor_add(gsum[:], gmax1[:], gmax2[:])

                # --- top-4 groups -> expert mask -> masked scores ---
                gsort = smp.tile([P, 8], f32, tag="gsort")
                nc.vector.max(out=gsort[:], in_=gsum[:])
                gmask = smp.tile([P, NG], f32, tag="gmask")
                nc.vector.tensor_scalar(
                    out=gmask[:], in0=gsum[:], scalar1=gsort[:, 3:4], scalar2=None,
                    op0=op.is_ge)
                tmp = sp.tile([P, EXP], f32, tag="tmp")
                nc.vector.tensor_tensor(
                    tmp[:].rearrange("p (g e) -> p g e", e=GS), s4c3,
                    gmask[:].to_broadcast([P, NG, GS]), op=op.mult)

                # --- top-8 over masked corrected scores ---
                v8 = outp.tile([P, 8], f32, tag="v8")
                nc.vector.max(out=v8[:], in_=tmp[:])
                idx8 = outp.tile([P, 8], mybir.dt.uint32, tag="idx8")
                nc.vector.max_index(out=idx8[:], in_max=v8[:], in_values=tmp[:])

                # --- gather uncorrected scores at the top-8 positions ---
                # keyed on the (unique) index, not the value: two experts can
                # have bitwise-equal corrected scores, which would double-match
                idxf = outp.tile([P, 8], f32, tag="idxf")
                nc.vector.tensor_copy(idxf[:], idx8[:])
                wsel = outp.tile([P, 8], f32, tag="wsel")
                scratch = sp.tile([P, EXP], f32, tag="scratch")
                for kk in range(8):
                    nc.vector.scalar_tensor_tensor(
                        out=scratch[:], in0=iota_t[:], scalar=idxf[:, kk:kk + 1],
                        in1=scores[:], op0=op.is_equal, op1=op.mult,
                        accum_out=wsel[:, kk:kk + 1])

                # --- renormalize * 2.5 ---
                denom = smp.tile([P, 1], f32, tag="denom")
                nc.vector.reduce_sum(denom[:], wsel[:], axis=X)
                nc.vector.tensor_scalar_add(denom[:], denom[:], 1e-20)
                recip = smp.tile([P, 1], f32, tag="recip")
                nc.vector.reciprocal(recip[:], denom[:])
                wfin = outp.tile([P, 8], f32, tag="wfin")
                nc.vector.tensor_scalar(
                    out=wfin[:], in0=wsel[:], scalar1=recip[:, 0:1], scalar2=SCALE,
                    op0=op.mult, op1=op.mult)

                nc.sync.dma_start(out=d_ow[t * P:(t + 1) * P, :], in_=wfin[:])
                nc.sync.dma_start(out=d_oi[t * P:(t + 1) * P, :],
                                  in_=idx8[:].bitcast(mybir.dt.int32))

    nc.compile()
    return nc


def _get_program():
    if "nc" not in _CACHE:
        _CACHE["nc"] = _build_program()
    return _CACHE["nc"]


def _prepare_in_maps(hidden_states, weight, e_score_correction_bias):
    h = np.asarray(hidden_states, dtype=np.float32)
    w = np.asarray(weight, dtype=np.float32)
    b = np.asarray(e_score_correction_bias, dtype=np.float32)

    bf16 = ml_dtypes.bfloat16
    hT = np.ascontiguousarray(h.T)                      # [HID, SEQ]
    h_hi = hT.astype(bf16)
    h_lo = (hT - h_hi.astype(np.float32)).astype(bf16)
    # pack to [p, T_global, k, s, m]: value = h_s[k*128+p, T*128+m]
    n_gt = SEQ // P                                     # 128 global tiles
    hcat = np.empty((P, n_gt, KT, 2, P), dtype=bf16)
    hcat[:, :, :, 0] = h_hi.reshape(KT, P, n_gt, P).transpose(1, 2, 0, 3)
    hcat[:, :, :, 1] = h_lo.reshape(KT, P, n_gt, P).transpose(1, 2, 0, 3)

    wT = np.ascontiguousarray(w.T)                      # [HID, EXP]
    w_hi = wT.astype(bf16)
    w_lo = (wT - w_hi.astype(np.float32)).astype(bf16)
    w_cat = np.concatenate([w_hi, w_lo], axis=1)        # [HID, 512]
    # pack to [p, k, e]
    w_cat = np.ascontiguousarray(
        w_cat.reshape(KT, P, 2 * EXP).transpose(1, 0, 2).reshape(P, KT * 2 * EXP))
    bias_rep = np.ascontiguousarray(np.broadcast_to(b[None, :], (P, EXP)))
    iota_rep = np.ascontiguousarray(
        np.broadcast_to(np.arange(EXP, dtype=np.float32)[None, :], (P, EXP)))

    in_maps = []
    for c in range(N_CORES):
        sl = slice(c * TILES, (c + 1) * TILES)
        in_maps.append({
            "h_cat": hcat[:, sl].reshape(P, TILES * 2 * KT * P),
            "w_cat": w_cat,
            "bias_rep": bias_rep,
            "iota_rep": iota_rep,
        })
    return in_maps


def kernel(hidden_states, weight, e_score_correction_bias):
    global LAST_RESULTS
    from concourse.bass_utils import run_bass_kernel_spmd

    nc = _get_program()
    in_maps = _prepare_in_maps(hidden_states, weight, e_score_correction_bias)

    trace = bool(int(os.environ.get("KERNEL_TRACE", "0")))
    res = run_bass_kernel_spmd(nc, in_maps, core_ids=list(range(N_CORES)),
                               trace=trace)
    LAST_RESULTS = res

    topk_w = np.concatenate([res.results[c]["out_w"] for c in range(N_CORES)], axis=0)
    topk_i = np.concatenate([res.results[c]["out_i"] for c in range(N_CORES)], axis=0)
    return topk_w, topk_i


# revision 6
# speedup vs baseline: 1.2491x; 1.0203x over previous
"""MoE gate (DeepSeek-V3 noaux_tc routing) on 8 Trainium2 NeuronCores.

Strategy: sequence-parallel — shard the 16384-token axis across 8 cores
(2048 tokens each), replicate the [256,7168] gate weight.

Numerics: router matmul done as 3-term bf16 split (hi*hi + hi*lo + lo*hi)
which measures ~5e-6 rms vs fp64 (close to fp32's own rounding noise).
fp32r was measured at 1.8e-4 rms — too coarse for top-k boundaries.
Top-k uses the DVE max8/max_index instructions (exact, stable-index).

v2: DRAM layout packed host-side as [partition][tile][hi/lo][k][m] so each
per-tile DMA is one contiguous 28KB run per partition (was 256B runs —
descriptor-rate-bound, stalling the PE). Weight DMA split in two so the
first k-chunks' matmuls start before the full weight lands.
"""
import sys
import os

sys.path.insert(0, "/opt/trn_rl_repo")

import numpy as np
import ml_dtypes

SEQ = 16384
HID = 7168
EXP = 256
N_CORES = 8
TOK = SEQ // N_CORES          # 2048 tokens per core
P = 128                       # partition dim / token tile
TILES = TOK // P              # 16
KT = HID // P                 # 56 contraction chunks
KHALF = KT // 2               # 28 (weight DMA split point)
NG = 8                        # groups
GS = EXP // NG                # 32 experts per group
SCALE = 2.5

_CACHE = {}
LAST_RESULTS = None


def _build_program():
    import concourse.mybir as mybir
    import concourse.tile as tile
    from concourse import bacc

    nc = bacc.Bacc("TRN2", target_bir_lowering=False, debug=False,
                   num_devices=N_CORES)

    bf16 = mybir.dt.bfloat16
    f32 = mybir.dt.float32

    # h_cat layout: [p][t][k][s][m], s=0 is bf16-hi, s=1 is bf16-lo
    # (s inside k so a k-range is one contiguous run per partition)
    d_hcat = nc.dram_tensor("h_cat", [P, TILES * 2 * KT * P], bf16,
                            kind="ExternalInput").ap()
    # w_cat layout: [p][k][e(512)], e 0:256 = w_hi, 256:512 = w_lo
    d_wcat = nc.dram_tensor("w_cat", [P, KT * 2 * EXP], bf16,
                            kind="ExternalInput").ap()
    d_bias = nc.dram_tensor("bias_rep", [P, EXP], f32, kind="ExternalInput").ap()
    d_iota = nc.dram_tensor("iota_rep", [P, EXP], f32, kind="ExternalInput").ap()
    d_ow = nc.dram_tensor("out_w", [TOK, 8], f32, kind="ExternalOutput").ap()
    d_oi = nc.dram_tensor("out_i", [TOK, 8], mybir.dt.int32, kind="ExternalOutput").ap()

    # [p, t, half(2), chunk-per-half] for the split h DMAs
    hcat_t = d_hcat.rearrange("p (t h r) -> p t h r", t=TILES, h=2)
    WCH = 8                       # k's per weight chunk
    NWCH = KT // WCH              # 7 weight chunks
    wcat_c = d_wcat.rearrange("p (c r) -> p c r", c=NWCH)

    X = mybir.AxisListType.X
    op = mybir.AluOpType

    with tile.TileContext(nc) as tc:
        with tc.tile_pool(name="wpool", bufs=1) as wp, \
             tc.tile_pool(name="hpool", bufs=3) as hp, \
             tc.tile_pool(name="spool", bufs=2) as sp, \
             tc.tile_pool(name="smalls", bufs=2) as smp, \
             tc.tile_pool(name="opool", bufs=3) as outp, \
             tc.tile_pool(name="psum", bufs=2, space="PSUM") as pp:

            # --- persistent weights / bias. DMAs drain in program order, so
            # interleave the first tile's h chunks between the w chunks: the
            # first matmuls need only w_c0 + h(t0,k<28) ≈ 2.8MB, not 11MB. ---
            w_c = []

            def _wchunk(c):
                wt = wp.tile([P, WCH * 2 * EXP], bf16, tag=f"w_c{c}")
                nc.sync.dma_start(out=wt[:], in_=wcat_c[:, c])
                w_c.append(wt)

            _wchunk(0)
            h0_a = hp.tile([P, KHALF * 2 * P], bf16, tag="hcat_a")
            nc.sync.dma_start(out=h0_a[:], in_=hcat_t[:, 0, 0])
            _wchunk(1)
            h0_b = hp.tile([P, KHALF * 2 * P], bf16, tag="hcat_b")
            nc.sync.dma_start(out=h0_b[:], in_=hcat_t[:, 0, 1])
            for c in range(2, NWCH):
                _wchunk(c)
            bias_t = wp.tile([P, EXP], f32, tag="bias")
            nc.sync.dma_start(out=bias_t[:], in_=d_bias)
            iota_t = wp.tile([P, EXP], f32, tag="iota")
            nc.sync.dma_start(out=iota_t[:], in_=d_iota)

            for t in range(TILES):
                # --- load hidden tile: two DMAs (k<28, k>=28), contiguous ---
                if t == 0:
                    hc_a, hc_b = h0_a, h0_b
                else:
                    hc_a = hp.tile([P, KHALF * 2 * P], bf16, tag="hcat_a")
                    nc.sync.dma_start(out=hc_a[:], in_=hcat_t[:, t, 0])
                    hc_b = hp.tile([P, KHALF * 2 * P], bf16, tag="hcat_b")
                    nc.sync.dma_start(out=hc_b[:], in_=hcat_t[:, t, 1])
                hc4a = hc_a[:].rearrange("p (k s m) -> p k s m", s=2, m=P)
                hc4b = hc_b[:].rearrange("p (k s m) -> p k s m", s=2, m=P)

                # --- matmul: psum[:, :256] = hi*w_hi (+ lo*w_hi),
                #             psum[:, 256:] = hi*w_lo ---
                ps = pp.tile([P, 2 * EXP], f32, tag="ps")
                for k in range(KT):
                    hck = hc4a if k < KHALF else hc4b
                    kk = k if k < KHALF else k - KHALF
                    wt = w_c[k // WCH]
                    ws = (k % WCH) * 2 * EXP
                    nc.tensor.matmul(
                        ps[:, :], hck[:, kk, 0], wt[:, ws:ws + 2 * EXP],
                        start=(k == 0), stop=False, skip_group_check=True)
                    nc.tensor.matmul(
                        ps[:, 0:EXP], hck[:, kk, 1], wt[:, ws:ws + EXP],
                        start=False, stop=(k == KT - 1), skip_group_check=True)

                # --- logits = left + right; scores = sigmoid(logits) ---
                # (DVE can read only one PSUM operand; bounce right half via ACT)
                xterm = sp.tile([P, EXP], f32, tag="xterm")
                nc.scalar.copy(xterm[:], ps[:, EXP:2 * EXP])
                logits = sp.tile([P, EXP], f32, tag="logits")
                nc.vector.tensor_add(logits[:], ps[:, 0:EXP], xterm[:])
                scores = sp.tile([P, EXP], f32, tag="scores")
                nc.scalar.activation(scores[:], logits[:],
                                     mybir.ActivationFunctionType.Sigmoid)
                s4c = sp.tile([P, EXP], f32, tag="s4c")
                nc.vector.tensor_add(s4c[:], scores[:], bias_t[:])
                s4c3 = s4c[:].rearrange("p (g e) -> p g e", e=GS)

                # --- group scores: sum of top-2 per group of 32 ---
                gmax1 = smp.tile([P, NG], f32, tag="gmax1")
                nc.vector.reduce_max(gmax1[:], s4c3, axis=X)
                eq = sp.tile([P, EXP], f32, tag="eq")
                nc.vector.tensor_tensor(
                    eq[:].rearrange("p (g e) -> p g e", e=GS), s4c3,
                    gmax1[:].to_broadcast([P, NG, GS]), op=op.is_equal)
                masked = sp.tile([P, EXP], f32, tag="masked")
                nc.vector.scalar_tensor_tensor(
                    out=masked[:], in0=eq[:], scalar=-1e30, in1=s4c[:],
                    op0=op.mult, op1=op.add)
                gmax2 = smp.tile([P, NG], f32, tag="gmax2")
                nc.vector.reduce_max(
                    gmax2[:], masked[:].rearrange("p (g e) -> p g e", e=GS), axis=X)
                gsum = smp.tile([P, NG], f32, tag="gsum")
                nc.vector.tensor_add(gsum[:], gmax1[:], gmax2[:])

                # --- top-4 groups -> expert mask -> masked scores ---
                gsort = smp.tile([P, 8], f32, tag="gsort")
                nc.vector.max(out=gsort[:], in_=gsum[:])
                gmask = smp.tile([P, NG], f32, tag="gmask")
                nc.vector.tensor_scalar(
                    out=gmask[:], in0=gsum[:], scalar1=gsort[:, 3:4], scalar2=None,
                    op0=op.is_ge)
                tmp = sp.tile([P, EXP], f32, tag="tmp")
                nc.vector.tensor_tensor(
                    tmp[:].rearrange("p (g e) -> p g e", e=GS), s4c3,
                    gmask[:].to_broadcast([P, NG, GS]), op=op.mult)

                # --- top-8 over masked corrected scores ---
                v8 = outp.tile([P, 8], f32, tag="v8")
                nc.vector.max(out=v8[:], in_=tmp[:])
                idx8 = outp.tile([P, 8], mybir.dt.uint32, tag="idx8")
                nc.vector.max_index(out=idx8[:], in_max=v8[:], in_values=tmp[:])

                # --- gather uncorrected scores at the top-8 positions ---
                # keyed on the (unique) index, not the value: two experts can
                # have bitwise-equal corrected scores, which would double-match
                idxf = outp.tile([P, 8], f32, tag="idxf")
                nc.vector.tensor_copy(idxf[:], idx8[:])
                wsel = outp.tile([P, 8], f32, tag="wsel")
                scratch = sp.tile([P, EXP], f32, tag="scratch")
                for kk in range(8):
                    nc.vector.scalar_tensor_tensor(
                        out=scratch[:], in0=iota_t[:], scalar=idxf[:, kk:kk + 1],
                        in1=scores[:], op0=op.is_equal, op1=op.mult,
                        accum_out=wsel[:, kk:kk + 1])

                # --- renormalize * 2.5 ---
                denom = smp.tile([P, 1], f32, tag="denom")
                nc.vector.reduce_sum(denom[:], wsel[:], axis=X)
                nc.vector.tensor_scalar_add(denom[:], denom[:], 1e-20)
                recip = smp.tile([P, 1], f32, tag="recip")
                nc.vector.reciprocal(recip[:], denom[:])
                wfin = outp.tile([P, 8], f32, tag="wfin")
                nc.vector.tensor_scalar(
                    out=wfin[:], in0=wsel[:], scalar1=recip[:, 0:1], scalar2=SCALE,
                    op0=op.mult, op1=op.mult)

                nc.sync.dma_start(out=d_ow[t * P:(t + 1) * P, :], in_=wfin[:])
                nc.sync.dma_start(out=d_oi[t * P:(t + 1) * P, :],
                                  in_=idx8[:].bitcast(mybir.dt.int32))

    nc.compile()
    return nc


def _get_program():
    if "nc" not in _CACHE:
        _CACHE["nc"] = _build_program()
    return _CACHE["nc"]


def _prepare_in_maps(hidden_states, weight, e_score_correction_bias):
    h = np.asarray(hidden_states, dtype=np.float32)
    w = np.asarray(weight, dtype=np.float32)
    b = np.asarray(e_score_correction_bias, dtype=np.float32)

    bf16 = ml_dtypes.bfloat16
    hT = np.ascontiguousarray(h.T)                      # [HID, SEQ]
    h_hi = hT.astype(bf16)
    h_lo = (hT - h_hi.astype(np.float32)).astype(bf16)
    # pack to [p, T_global, k, s, m]: value = h_s[k*128+p, T*128+m]
    n_gt = SEQ // P                                     # 128 global tiles
    hcat = np.empty((P, n_gt, KT, 2, P), dtype=bf16)
    hcat[:, :, :, 0] = h_hi.reshape(KT, P, n_gt, P).transpose(1, 2, 0, 3)
    hcat[:, :, :, 1] = h_lo.reshape(KT, P, n_gt, P).transpose(1, 2, 0, 3)

    wT = np.ascontiguousarray(w.T)                      # [HID, EXP]
    w_hi = wT.astype(bf16)
    w_lo = (wT - w_hi.astype(np.float32)).astype(bf16)
    w_cat = np.concatenate([w_hi, w_lo], axis=1)        # [HID, 512]
    # pack to [p, k, e]
    w_cat = np.ascontiguousarray(
        w_cat.reshape(KT, P, 2 * EXP).transpose(1, 0, 2).reshape(P, KT * 2 * EXP))
    bias_rep = np.ascontiguousarray(np.broadcast_to(b[None, :], (P, EXP)))
    iota_rep = np.ascontiguousarray(
        np.broadcast_to(np.arange(EXP, dtype=np.float32)[None, :], (P, EXP)))

    in_maps = []
    for c in range(N_CORES):
        sl = slice(c * TILES, (c + 1) * TILES)
        in_maps.append({
            "h_cat": hcat[:, sl].reshape(P, TILES * 2 * KT * P),
            "w_cat": w_cat,
            "bias_rep": bias_rep,
            "iota_rep": iota_rep,
        })
    return in_maps


def kernel(hidden_states, weight, e_score_correction_bias):
    global LAST_RESULTS
    from concourse.bass_utils import run_bass_kernel_spmd

    nc = _get_program()
    in_maps = _prepare_in_maps(hidden_states, weight, e_score_correction_bias)

    trace = bool(int(os.environ.get("KERNEL_TRACE", "0")))
    res = run_bass_kernel_spmd(nc, in_maps, core_ids=list(range(N_CORES)),
                               trace=trace)
    LAST_RESULTS = res

    topk_w = np.concatenate([res.results[c]["out_w"] for c in range(N_CORES)], axis=0)
    topk_i = np.concatenate([res.results[c]["out_i"] for c in range(N_CORES)], axis=0)
    return topk_w, topk_i
